# revision 55
# baseline (speedup 1.0000x reference)
"""AttentionBlock (GroupNorm + 8-head self-attention + proj + residual) on 8 trn2 cores.

Sharding: data-parallel over batch B=16 -> 2 samples per core. No collectives.

v2 (per-head bf16 pipeline):
  - All PE matmul operands are bf16 (hardware runs fp32r as two half-rate
    passes; bf16 is single-pass and halves LDWEIGHTS size). PSUM stays fp32.
  - Weights arrive HOST-pretransposed and pre-cast: wT = qkv_w.T (bf16),
    pT = proj_w.T (bf16), plus precomputed groupnorm/selector masks and
    bias columns - no on-device transposes or gpsimd mask building.
  - GroupNorm per sample: bn_stats over L, 16-channel group aggregation +
    broadcast-back via tiny mask matmuls on the PE (fp32, negligible).
  - QKV: q^T,k^T as (channels, L) bf16 tiles; v in (L, channels) orientation
    with a ones column appended (M=65) so the softmax denominator rides in
    PSUM row 64 of the AV matmul.
  - Attention PER HEAD with full-L matmuls: S^T chunk = kT.T @ qT (K=64,
    N=1024), exp on ScalarE (scale 1/8 fused) -> bf16 e tile, AV accumulates
    v'[jc].T @ e over the 8 j-chunks into one (65, 1024) PSUM tile.
    Attention row copies + denominator staging drain on GpSimd.
  - Denominators collect in one (8, L) tile per sample; per head-pair slices
    get reciprocal_approx_fast + bf16 cast, then a K=2 selector matmul
    broadcasts them to channel rows for one normalization multiply.
  - proj + bias + residual (x stays resident in SBUF), write out.
  - Cross-sample software pipeline: filler units (qkv/v of the next sample,
    proj of the previous) pop one per attention step to soak the PE while
    ScalarE works through exps.
"""

import numpy as np
import ml_dtypes

import concourse.bass as bass
import concourse.mybir as mybir
import concourse.tile as tile
from concourse import bacc
from concourse.bass_utils import run_bass_kernel_spmd

F32 = mybir.dt.float32
BF16 = mybir.dt.bfloat16
FP8 = mybir.dt.float8e4
DR = mybir.MatmulPerfMode.DoubleRow
EXP_BIAS = -2.0  # exp(s-2): keeps e in fp8e4m3 range; cancels in the softmax ratio
AF = mybir.ActivationFunctionType
OP = mybir.AluOpType

B, C, H, W = 16, 512, 32, 32
L = H * W
NH, HD = 8, 64
NG, GS = 32, 16
EPS = 1e-5
N_CORES = 8
BPC = B // N_CORES  # samples per core
P = 128
CK = C // P   # 4 channel chunks
LK = L // P   # 8 pixel chunks
SCALE = HD ** -0.5

_NC_CACHE = {}


class Ctx:
    pass


def _consts(nc, const, wT_d, pT_d, masks_d, bcols_d, bmask_d, sel2_d, prow_d):
    c_prow_ap = prow_d.ap()
    """Emit const DMAs in deadline order: small gn masks first, then the
    weight columns the prologue touches (q0/k0 chunks, v), then the rest.
    pT is deferred to _consts_late (first needed mid-attention)."""
    c = Ctx()

    # masks layout: [vb (512) | gmask (4*32) | ones (8)]
    masks = const.tile([P, 512 + CK * NG + NH], F32, tag="masks", name="masks")
    nc.sync.dma_start(masks, masks_d.ap())
    c.vb = masks[:, 0:512]
    c.gmask = [masks[:, 512 + kc * NG: 512 + (kc + 1) * NG] for kc in range(CK)]
    c.ones_col = masks[:, 512 + CK * NG: 512 + CK * NG + NH]

    # bcols layout: [nw (4) | nb (4) | pb (4) | qb (8)]
    bcols = const.tile([P, 20], F32, tag="bcols", name="bcols")
    nc.sync.dma_start(bcols, bcols_d.ap())
    c.nw = [bcols[:, kc: kc + 1] for kc in range(CK)]
    c.nb = [bcols[:, 4 + kc: 5 + kc] for kc in range(CK)]
    c.nw_all = bcols[:, 0:CK]
    c.nb_all = bcols[:, 4:4 + CK]
    c.pb = [bcols[:, 8 + kc: 9 + kc] for kc in range(CK)]
    c.qb = [bcols[:, 12 + oc: 13 + oc] for oc in range(8)]

    c.bmask = const.tile([NG, C], F32, tag="bmask", name="bmask")
    nc.sync.dma_start(c.bmask, bmask_d.ap())
    c.sel2 = const.tile([2, P], BF16, tag="sel2", name="sel2")
    nc.sync.dma_start(c.sel2, sel2_d.ap())
    c.pbrow = const.tile([1, C], BF16, tag="pbrow", name="pbrow")
    nc.sync.dma_start(c.pbrow, c_prow_ap[0:1])
    c.onesrow = const.tile([1, C], BF16, tag="onesrow", name="onesrow")
    nc.sync.dma_start(c.onesrow, c_prow_ap[1:2])
    c.eps_t = const.tile([NG, 1], F32, tag="eps_t")
    nc.vector.memset(c.eps_t, EPS)
    c.ebias = const.tile([P, 1], F32, tag="ebias")
    nc.vector.memset(c.ebias, EXP_BIAS)

    wT_r = wT_d.ap().rearrange("(kc p) o -> kc p o", p=P)
    c.wT = [const.tile([P, 3 * C], BF16, tag=f"wT{kc}", name=f"wT{kc}")
            for kc in range(CK)]
    for kc in range(CK):   # whole tile: contiguous 3KB rows DMA efficiently
        nc.sync.dma_start(c.wT[kc], wT_r[kc])
    c.pT_d = pT_d
    return c


def _consts_late(nc, const, c):
    pT_r = c.pT_d.ap().rearrange("(kc p) o -> kc p o", p=P)
    c.pT = []
    for kc in range(CK):
        t = const.tile([P, C], BF16, tag=f"pT{kc}", name=f"pT{kc}")
        nc.sync.dma_start(t, pT_r[kc])
        c.pT.append(t)


def _emit(nc, tc, pools, c_box, const, x_d, out_d, wT_d, pT_d, masks_d,
          bcols_d, bmask_d, sel2_d, prow_d):
    xp, hp_, qkp, vp, ep, attp, op_, sm, csp, ps, ps2 = pools

    x_r = x_d.ap().rearrange("b (kc p) h w -> b kc p (h w)", p=P)
    o_r = out_d.ap().rearrange("b (kc p) h w -> b kc p (h w)", p=P)

    S = [Ctx() for _ in range(BPC)]

    def emit_x_dma(s):
        st_ = S[s]
        st_.x = []
        for kc in range(CK):
            xt = xp.tile([P, L], F32, tag=f"x{kc}", name=f"x{kc}_{s}")
            nc.sync.dma_start(xt[:, 0:512], x_r[s, kc][:, 0:512])
            nc.sync.dma_start(xt[:, 512:1024], x_r[s, kc][:, 512:1024])
            st_.x.append(xt)
        st_.stat2 = [None] * CK

    def emit_gn_stats_kc(s, kc):
        st_ = S[s]
        xt = st_.x[kc]
        bst = sm.tile([P, 2, 6], F32, tag="bst", name="bst")
        nc.vector.bn_stats(out=bst[:, 0, :], in_=xt[:, 0:512])
        nc.vector.bn_stats(out=bst[:, 1, :], in_=xt[:, 512:1024])
        mv = sm.tile([P, 2], F32, tag="mv", name="mv")
        nc.vector.bn_aggr(out=mv, in_=bst)
        st2 = sm.tile([P, 2], F32, tag="st2", name="st2")
        nc.vector.tensor_copy(out=st2[:, 0:1], in_=mv[:, 0:1])
        nc.vector.tensor_tensor(st2[:, 1:2], mv[:, 0:1], mv[:, 0:1], OP.mult)
        nc.vector.tensor_tensor(st2[:, 1:2], st2[:, 1:2], mv[:, 1:2], OP.add)
        st_.stat2[kc] = st2

    c = c_box

    def emit_gn_head(s):
        st_ = S[s]
        gps = ps2.tile([P, 512], F32, tag="p2", name="gn_ps")
        for kc in range(CK):
            nc.tensor.matmul(gps[0:NG, 0:2], c.gmask[kc], st_.stat2[kc],
                             start=(kc == 0), stop=(kc == CK - 1))
        gst = sm.tile([NG, 2], F32, tag="gst", name=f"gst_{s}")
        gsb = sm.tile([NG, 2], F32, tag="gsb", name="gsb")
        gtmp = sm.tile([NG, 1], F32, tag="gtmp", name="gtmp")
        nc.vector.tensor_copy(out=gsb, in_=gps[0:NG, 0:2])
        nc.vector.tensor_tensor(gtmp, gsb[:, 0:1], gsb[:, 0:1], OP.mult)
        nc.vector.tensor_tensor(gtmp, gsb[:, 1:2], gtmp, OP.subtract)  # var
        nc.scalar.activation(gtmp, gtmp, AF.Ln, bias=c.eps_t)
        nc.scalar.activation(gst[:, 1:2], gtmp, AF.Exp, scale=-0.5)    # rstd
        nc.vector.tensor_copy(out=gst[:, 0:1], in_=gsb[:, 0:1])        # gmean
        chps = ps2.tile([P, 512], F32, tag="p2", name="gn_ps2")
        for kc in range(CK):
            nc.tensor.matmul(chps[:, kc * 2: kc * 2 + 2],
                             c.bmask[:, kc * P:(kc + 1) * P], gst,
                             start=True, stop=True)
        st_.chps = chps
        # batched scale/shift columns for all 4 channel chunks: one strided
        # TT triple instead of 12 tiny DVE ops
        ch2 = chps[:, 0:2 * CK].rearrange("p (kc two) -> p two kc", two=2)
        Acols = sm.tile([P, CK], F32, tag="Acols", name=f"Acols_{s}")
        Bcols = sm.tile([P, CK], F32, tag="Bcols", name=f"Bcols_{s}")
        nc.vector.tensor_tensor(Acols, ch2[:, 1, :], c.nw_all, OP.mult)
        nc.vector.tensor_tensor(Bcols, ch2[:, 0, :], Acols, OP.mult)
        nc.vector.tensor_tensor(Bcols, c.nb_all, Bcols, OP.subtract)
        st_.Acols, st_.Bcols = Acols, Bcols
        st_.qkT = [None] * 8
        st_.v = [None] * LK
        st_.att = [None] * CK
        st_.csum = [None] * CK

    def emit_gn_h_kc(s, kc):
        st_ = S[s]
        ht = hp_.tile([P, L], BF16, tag=f"h{kc}", name=f"h{kc}_{s}")
        # pure-SBUF op; split across DVE and GpSimd so the 4 chunks finish
        # in ~2 engine-slots instead of 4
        eng = nc.vector if kc < 2 else nc.gpsimd
        eng.tensor_scalar(ht, st_.x[kc], st_.Acols[:, kc:kc + 1],
                          st_.Bcols[:, kc:kc + 1], op0=OP.mult, op1=OP.add)
        st_.h[kc] = ht

    def emit_gn_apply(s):
        S[s].h = [None] * CK
        emit_gn_head(s)
        for kc in range(CK):
            emit_gn_h_kc(s, kc)

    def emit_qkv_unit(s, oc, li):
        st_ = S[s]
        if st_.qkT[oc] is None:
            st_.qkT[oc] = qkp.tile([P, L], BF16, tag=f"qk{oc}", name=f"qk{oc}_{s}")
        dst = st_.qkT[oc]
        pt = ps2.tile([P, 512], F32, tag="p2", name="qkv_ps")
        for kc in range(CK):
            nc.tensor.matmul(pt,
                             c.wT[kc][:, oc * P:(oc + 1) * P],
                             st_.h[kc][:, li * 512:(li + 1) * 512],
                             start=(kc == 0), stop=(kc == CK - 1))
        nc.vector.tensor_scalar(dst[:, li * 512:(li + 1) * 512],
                                pt, c.qb[oc], None, op0=OP.add)

    def emit_qkv_qk(s, hp):
        for li in range(2):
            for oc in (hp, 4 + hp):
                emit_qkv_unit(s, oc, li)

    def emit_v(s, lcs):
        st_ = S[s]
        for lc in lcs:
            pt = ps2.tile([P, 512], F32, tag="p2", name="v_ps")
            for kc in range(CK):
                nc.tensor.matmul(pt,
                                 st_.h[kc][:, lc * P:(lc + 1) * P],
                                 c.wT[kc][:, 1024:1536],
                                 start=(kc == 0), stop=(kc == CK - 1))
            vt = vp.tile([P, NH, HD + 1], BF16, tag=f"v{lc}", name=f"v{lc}_{s}")
            nc.vector.tensor_copy(out=vt[:, :, HD:HD + 1],
                                  in_=c.ones_col[:, :, None])
            nc.vector.tensor_tensor(
                vt[:, :, 0:HD],
                pt.rearrange("p (h d) -> p h d", d=HD),
                c.vb.rearrange("p (h d) -> p h d", d=HD),
                OP.add)
            st_.v[lc] = vt

    fill_q = []

    def pop_fill():
        if fill_q:
            fill_q.pop(0)()

    def at_ms(ms, f):
        def g():
            with tc.tile_wait_until(ms):
                f()
        return g

    def make_norm2(s, hp, rsum):
        st_ = S[s]

        def norm2():
            for li in range(2):
                rb2 = ps2.tile([P, 512], F32, tag="p2", name="rb2_ps")
                nc.tensor.matmul(rb2, c.sel2, rsum[:, li * 512:(li + 1) * 512],
                                 start=True, stop=True)
                nc.vector.tensor_tensor(
                    st_.att[hp][:, li * 512:(li + 1) * 512],
                    st_.att[hp][:, li * 512:(li + 1) * 512], rb2, OP.mult)
        return norm2

    def emit_head(s, h):
        st_ = S[s]
        hp, h2 = h // 2, h % 2
        kT, qT = st_.qkT[4 + hp], st_.qkT[hp]
        if st_.att[hp] is None:
            st_.att[hp] = attp.tile([P, L], BF16, tag=f"att{hp}", name=f"att{hp}_{s}")
            st_.csum[hp] = csp.tile([2, L], F32, tag=f"csum{hp}",
                                    name=f"csum{hp}_{s}", bufs=1)
        av = ps.tile([P, L], F32, tag="s", name=f"av_{s}_{h}")

        def s_mm(jc):
            # matmul outputs may not cross a PSUM bank (512 f32), so the
            # full-L S^T chunk is two N=512 matmuls into one 2-bank tile
            stile = ps.tile([P, L], F32, tag="s", name=f"s_{s}_{h}_{jc}")
            for ih in range(2):
                nc.tensor.matmul(stile[:, ih * 512:(ih + 1) * 512],
                                 kT[h2 * HD:(h2 + 1) * HD, jc * P:(jc + 1) * P],
                                 qT[h2 * HD:(h2 + 1) * HD, ih * 512:(ih + 1) * 512],
                                 start=True, stop=True)
            return stile

        # attention chain emits at high priority: the scheduler prefers
        # S/exp/AV over filler work whenever both are ready, so fillers
        # backfill stalls instead of delaying the exp pace-setter
        with tc.high_priority(offset=1 << 20):
            stile = s_mm(0)
        for jc in range(LK):
            with tc.high_priority(offset=1 << 20):
                e_t = ep.tile([P, L], BF16, tag="e", name="e_t")
                nc.scalar.activation(e_t, stile, AF.Exp, scale=SCALE)
                # emit next S ahead of this AV so the PE stream runs one
                # step ahead of ScalarE
                if jc + 1 < LK:
                    stile = s_mm(jc + 1)
            pop_fill()
            with tc.high_priority(offset=1 << 20):
                for ih in range(2):
                    nc.tensor.matmul(av[0:HD + 1, ih * 512:(ih + 1) * 512],
                                     st_.v[jc][:, h, :],
                                     e_t[:, ih * 512:(ih + 1) * 512],
                                     start=(jc == 0), stop=(jc == LK - 1))
        cstage = csp.tile([1, L], F32, tag="cstage", name="cstage", bufs=2)
        nc.vector.tensor_copy(out=cstage, in_=av[HD:HD + 1, :])
        nc.sync.dma_start(st_.csum[hp][h2:h2 + 1, :], cstage)
        nc.vector.tensor_copy(out=st_.att[hp][h2 * HD:(h2 + 1) * HD, :],
                              in_=av[0:HD, :])
        if h2 == 1:
            # both heads of pair hp drained: reciprocal + bf16 cast
            rtmp = csp.tile([2, L], F32, tag="rtmp", name=f"rtmp_{s}_{hp}")
            rsum = csp.tile([2, L], BF16, tag=f"rsum{hp}",
                            name=f"rsum{hp}_{s}", bufs=1)
            nc.vector.reciprocal_approx_fast(out=rtmp, in_=st_.csum[hp])
            with nc.allow_low_precision(reason="bf16 rounding"):
                nc.vector.tensor_copy(out=rsum, in_=rtmp)
            fill_q.insert(min(len(fill_q), 4), make_norm2(s, hp, rsum))

    def emit_proj_li(s, oc, li):
        st_ = S[s]
        sl = slice(li * 512, (li + 1) * 512)
        pt = ps2.tile([P, 512], F32, tag="p2", name="proj_ps")
        nc.tensor.matmul(pt, c.pbrow[0:1, oc * P:(oc + 1) * P],
                         c.onesrow[0:1, 0:512], start=True, stop=False)
        for kc in range(CK):
            nc.tensor.matmul(pt,
                             c.pT[kc][:, oc * P:(oc + 1) * P],
                             st_.att[kc][:, sl],
                             start=False, stop=(kc == CK - 1))
        ot = op_.tile([P, 512], F32, tag="otl", name="otl")
        nc.vector.tensor_tensor(ot, pt, st_.x[oc][:, sl], OP.add)
        nc.sync.dma_start(o_r[s, oc][:, sl], ot)

    def emit_proj_oc(s, oc, split_dma=False):
        st_ = S[s]
        ot = op_.tile([P, L], F32, tag="ot", name="ot")
        for li in range(2):
            sl = slice(li * 512, (li + 1) * 512)
            pt = ps2.tile([P, 512], F32, tag="p2", name="proj_ps")
            # bias folded into the accumulation: pb_row (x) ones_row
            nc.tensor.matmul(pt, c.pbrow[0:1, oc * P:(oc + 1) * P],
                             c.onesrow[0:1, 0:512], start=True, stop=False)
            for kc in range(CK):
                nc.tensor.matmul(pt,
                                 c.pT[kc][:, oc * P:(oc + 1) * P],
                                 st_.att[kc][:, sl],
                                 start=False, stop=(kc == CK - 1))
            nc.vector.tensor_tensor(ot[:, sl], pt, st_.x[oc][:, sl], OP.add)
            if split_dma:
                nc.sync.dma_start(o_r[s, oc][:, sl], ot[:, sl])
        if not split_dma:
            nc.sync.dma_start(o_r[s, oc], ot)

    # ---------------- schedule ----------------
    emit_x_dma(0)             # x(s0) DMAs lead the queue
    cc = _consts(nc, const, wT_d, pT_d, masks_d, bcols_d, bmask_d, sel2_d,
                 prow_d)
    c.__dict__.update(cc.__dict__)
    for kc in range(CK):
        emit_gn_stats_kc(0, kc)
    emit_gn_apply(0)
    emit_qkv_qk(0, 0)         # head 0/1 q,k
    emit_x_dma(1)             # x(s1) DMAs queue behind the weight loads
    _consts_late(nc, const, c)

    # everything else becomes filler units popped one per attention step; the
    # queue order encodes just-in-time deadlines (v2(0,jcp) pops before the
    # AV pair that consumes it; qkv(0,hp) before head 2hp's first S). Sample-1
    # groupnorm runs as fillers so its DVE chain stays off the prologue.
    for lc in range(LK):
        fill_q.append(lambda lc=lc: emit_v(0, [lc]))
    for oc in (1, 5):
        for li in range(2):
            fill_q.append(lambda oc=oc, li=li: emit_qkv_unit(0, oc, li))
    for oc in (2, 6, 3, 7):
        for li in range(2):
            fill_q.append(lambda oc=oc, li=li: emit_qkv_unit(0, oc, li))
    for kc in range(CK):
        fill_q.append(lambda kc=kc: emit_gn_stats_kc(1, kc))
    fill_q.append(lambda: emit_gn_head(1))
    for kc in range(CK):
        fill_q.append(lambda kc=kc: emit_gn_h_kc(1, kc))
    for oc in (0, 4):
        for li in range(2):
            fill_q.append(lambda oc=oc, li=li: emit_qkv_unit(1, oc, li))
    for lc in range(LK):
        fill_q.append(lambda lc=lc: emit_v(1, [lc]))
    for oc in (1, 5):
        for li in range(2):
            fill_q.append(lambda oc=oc, li=li: emit_qkv_unit(1, oc, li))
    fill_q.extend([lambda: None] * 10)
    for oc in (2, 6):
        for li in range(2):
            fill_q.append(lambda oc=oc, li=li: emit_qkv_unit(1, oc, li))
    fill_q.extend([lambda: None] * 14)
    for oc in (3, 7):
        for li in range(2):
            fill_q.append(lambda oc=oc, li=li: emit_qkv_unit(1, oc, li))

    # interleave the two samples' head loops: sample-1 S/AV becomes the PE
    # filler for sample-0's exp-paced steps and vice versa, keeping the PE
    # continuously warm (HAM re-throttles on idle gaps). All h(s0) readers
    # (v(0,*), qkv(0,*)) pop before gn(1)'s h writes - else PE<->DVE cycle.
    S[1].h = [None] * CK
    seq = [(0, 0), (0, 1), (0, 2), (0, 3), (0, 4), (1, 0), (0, 5), (1, 1),
           (0, 6), (1, 2), (0, 7), (1, 3), (1, 4), (1, 5), (1, 6), (1, 7)]
    for s, h in seq:
        emit_head(s, h)
        if (s, h) == (0, 7):
            fill_q.extend([lambda: None] * 12)
            for oc in range(CK):
                for li in range(2):
                    fill_q.append(lambda oc=oc, li=li: emit_proj_li(0, oc, li))
                    fill_q.append(lambda: None)
    while fill_q:
        pop_fill()
    for oc in range(CK):
        for li in range(2):
            emit_proj_li(1, oc, li)


def _build():
    if "nc" in _NC_CACHE:
        return _NC_CACHE["nc"]
    nc = bacc.Bacc("TRN2", target_bir_lowering=False, debug=False)
    x_d = nc.dram_tensor("x", (BPC, C, H, W), F32, kind="ExternalInput")
    wT_d = nc.dram_tensor("wT", (C, 3 * C), BF16, kind="ExternalInput")
    pT_d = nc.dram_tensor("pT", (C, C), BF16, kind="ExternalInput")
    masks_d = nc.dram_tensor("masks", (P, 512 + CK * NG + NH), F32,
                             kind="ExternalInput")
    bcols_d = nc.dram_tensor("bcols", (P, 20), F32, kind="ExternalInput")
    bmask_d = nc.dram_tensor("bmask", (NG, C), F32, kind="ExternalInput")
    sel2_d = nc.dram_tensor("sel2", (2, P), BF16, kind="ExternalInput")
    prow_d = nc.dram_tensor("prow", (2, C), BF16, kind="ExternalInput")
    out_d = nc.dram_tensor("out", (BPC, C, H, W), F32, kind="ExternalOutput")
    with tile.TileContext(nc) as tc:
        with (
            tc.tile_pool(name="const", bufs=1) as const,
            tc.tile_pool(name="xp", bufs=2) as xp,
            tc.tile_pool(name="hp", bufs=1) as hp_,
            tc.tile_pool(name="qkp", bufs=2) as qkp,
            tc.tile_pool(name="vp", bufs=2) as vp,
            tc.tile_pool(name="ep", bufs=3) as ep,
            tc.tile_pool(name="attp", bufs=2) as attp,
            tc.tile_pool(name="op", bufs=2) as op_,
            tc.tile_pool(name="sm", bufs=1) as sm,
            tc.tile_pool(name="csp", bufs=2) as csp,
            tc.tile_pool(name="ps", bufs=3, space="PSUM") as ps,
            tc.tile_pool(name="ps2", bufs=2, space="PSUM") as ps2,
        ):
            pools = (xp, hp_, qkp, vp, ep, attp, op_, sm, csp, ps, ps2)
            _emit(nc, tc, pools, Ctx(), const, x_d, out_d, wT_d, pT_d,
                  masks_d, bcols_d, bmask_d, sel2_d, prow_d)
    nc.compile()
    _NC_CACHE["nc"] = nc
    return nc


def _host_consts(norm_w, norm_b, qkv_w, qkv_b, proj_w, proj_b):
    bf16 = ml_dtypes.bfloat16
    wT = np.ascontiguousarray(qkv_w.T).astype(bf16)
    pT = np.ascontiguousarray(proj_w.T).astype(bf16)

    masks = np.zeros((P, 512 + CK * NG + NH), np.float32)
    masks[:, 0:512] = qkv_b[1024:1536][None, :]                      # vb
    for kc in range(CK):
        for p in range(P):
            g = (kc * P + p) // GS
            masks[p, 512 + kc * NG + g] = 1.0 / GS                   # gmask
    masks[:, 512 + CK * NG:] = 1.0                                   # ones

    bcols = np.zeros((P, 20), np.float32)
    for kc in range(CK):
        bcols[:, kc] = norm_w[kc * P:(kc + 1) * P]
        bcols[:, 4 + kc] = norm_b[kc * P:(kc + 1) * P]
        bcols[:, 8 + kc] = proj_b[kc * P:(kc + 1) * P]
    for oc in range(8):
        bcols[:, 12 + oc] = qkv_b[oc * P:(oc + 1) * P]

    bmask = np.zeros((NG, C), np.float32)
    for g in range(NG):
        bmask[g, g * GS:(g + 1) * GS] = 1.0

    sel2 = np.zeros((2, P), np.float32)
    sel2[0, 0:HD] = 1.0
    sel2[1, HD:P] = 1.0

    prow = np.ones((2, C), np.float32)
    prow[0] = proj_b

    return {"wT": wT, "pT": pT, "masks": masks, "bcols": bcols,
            "bmask": bmask, "sel2": sel2.astype(bf16),
            "prow": prow.astype(bf16)}


def make_in_maps(x, norm_w, norm_b, qkv_w, qkv_b, proj_w, proj_b):
    x = np.ascontiguousarray(x, dtype=np.float32)
    args = _host_consts(
        np.asarray(norm_w, np.float32), np.asarray(norm_b, np.float32),
        np.ascontiguousarray(qkv_w, np.float32), np.asarray(qkv_b, np.float32),
        np.ascontiguousarray(proj_w, np.float32), np.asarray(proj_b, np.float32))
    return [dict(args, x=x[i * BPC:(i + 1) * BPC]) for i in range(N_CORES)]


def kernel(x, norm_w, norm_b, qkv_w, qkv_b, proj_w, proj_b):
    nc = _build()
    in_maps = make_in_maps(x, norm_w, norm_b, qkv_w, qkv_b, proj_w, proj_b)
    res = run_bass_kernel_spmd(nc, in_maps, core_ids=list(range(N_CORES)))
    return np.concatenate([r["out"] for r in res.results], axis=0)


# revision 57
# speedup vs baseline: 1.1347x; 1.1347x over previous
"""AttentionBlock (GroupNorm + 8-head self-attention + proj + residual) on 8 trn2 cores.

Sharding: data-parallel over batch B=16 -> 2 samples per core. No collectives.

v2 (per-head bf16 pipeline, cross-sample interleaved):
  - All PE matmul operands are bf16 (hardware runs fp32r as two half-rate
    LOW_HIGH passes; bf16 is single-pass and halves LDWEIGHTS). PSUM fp32.
  - Weights arrive HOST-pretransposed and pre-cast: wT = qkv_w.T (bf16),
    pT = proj_w.T (bf16), plus precomputed groupnorm/selector masks and
    bias columns - no on-device transposes or gpsimd mask building.
  - GroupNorm per sample: bn_stats over L, 16-channel group aggregation +
    broadcast-back via tiny mask matmuls on the PE (fp32, negligible);
    normalized h written by DVE/GpSimd tensor_scalar (bf16).
  - QKV: q^T,k^T as (channels, L) bf16 tiles; v in (L, channels) orientation
    with a ones column appended (M=65) so the softmax denominator rides in
    PSUM row 64 of the AV matmul.
  - Attention PER HEAD: per j-chunk, S^T = kT.T @ qT as two N=512 matmuls
    into one 2-bank PSUM tile (matmul out must not cross a PSUM bank), ONE
    [128,1024] exp on ScalarE (scale 1/8 fused) -> bf16 e tile, AV
    accumulates v'[jc].T @ e into a (65, 1024) PSUM tile.
  - Per pair: denominators to a (2, L) tile (DVE copy + DMA), then
    reciprocal_approx_fast + bf16 cast; a K=2 selector matmul broadcasts
    them to channel rows for one normalization multiply.
  - proj with bias folded into the accumulation as a rank-1 matmul,
    residual from SBUF-resident x, write out.
  - The two samples' head loops INTERLEAVE (s0 h0-4, then alternating),
    so each sample's S/AV fills the other's exp-paced stalls: the PE stays
    continuously busy and HAM keeps the clock up. Remaining work (qkv/v of
    sample 1, proj of sample 0, sample-1 groupnorm) pops one unit per
    attention step from a deadline-ordered filler queue with spacers to
    reach the otherwise-starved late steps.
"""

import numpy as np
import ml_dtypes

import concourse.bass as bass
import concourse.mybir as mybir
import concourse.tile as tile
from concourse import bacc
from concourse.bass_utils import run_bass_kernel_spmd

F32 = mybir.dt.float32
BF16 = mybir.dt.bfloat16
FP8 = mybir.dt.float8e4
DR = mybir.MatmulPerfMode.DoubleRow
EXP_BIAS = -2.0  # exp(s-2): keeps e in fp8e4m3 range; cancels in the softmax ratio
AF = mybir.ActivationFunctionType
OP = mybir.AluOpType

B, C, H, W = 16, 512, 32, 32
L = H * W
NH, HD = 8, 64
NG, GS = 32, 16
EPS = 1e-5
N_CORES = 8
BPC = B // N_CORES  # samples per core
P = 128
CK = C // P   # 4 channel chunks
LK = L // P   # 8 pixel chunks
SCALE = HD ** -0.5

_NC_CACHE = {}


class Ctx:
    pass


def _consts(nc, const, wT_d, pT_d, masks_d, bcols_d, bmask_d, sel2_d, prow_d):
    c_prow_ap = prow_d.ap()
    """Emit const DMAs in deadline order: small gn masks first, then the
    weight columns the prologue touches (q0/k0 chunks, v), then the rest.
    pT is deferred to _consts_late (first needed mid-attention)."""
    c = Ctx()

    # masks layout: [vb (512) | gmask (4*32) | ones (8)]
    masks = const.tile([P, 512 + CK * NG + NH], F32, tag="masks", name="masks")
    nc.sync.dma_start(masks, masks_d.ap())
    c.vb = masks[:, 0:512]
    c.gmask = [masks[:, 512 + kc * NG: 512 + (kc + 1) * NG] for kc in range(CK)]
    c.ones_col = masks[:, 512 + CK * NG: 512 + CK * NG + NH]

    # bcols layout: [nw (4) | nb (4) | pb (4) | qb (8)]
    bcols = const.tile([P, 20], F32, tag="bcols", name="bcols")
    nc.sync.dma_start(bcols, bcols_d.ap())
    c.nw = [bcols[:, kc: kc + 1] for kc in range(CK)]
    c.nb = [bcols[:, 4 + kc: 5 + kc] for kc in range(CK)]
    c.nw_all = bcols[:, 0:CK]
    c.nb_all = bcols[:, 4:4 + CK]
    c.pb = [bcols[:, 8 + kc: 9 + kc] for kc in range(CK)]
    c.qb = [bcols[:, 12 + oc: 13 + oc] for oc in range(8)]

    c.bmask = const.tile([NG, C], F32, tag="bmask", name="bmask")
    nc.sync.dma_start(c.bmask, bmask_d.ap())
    c.sel2 = const.tile([2, P], BF16, tag="sel2", name="sel2")
    nc.sync.dma_start(c.sel2, sel2_d.ap())
    c.pbrow = const.tile([1, C], BF16, tag="pbrow", name="pbrow")
    nc.sync.dma_start(c.pbrow, c_prow_ap[0:1])
    c.onesrow = const.tile([1, C], BF16, tag="onesrow", name="onesrow")
    nc.sync.dma_start(c.onesrow, c_prow_ap[1:2])
    c.eps_t = const.tile([NG, 1], F32, tag="eps_t")
    nc.vector.memset(c.eps_t, EPS)
    c.ebias = const.tile([P, 1], F32, tag="ebias")
    nc.vector.memset(c.ebias, EXP_BIAS)

    wT_r = wT_d.ap().rearrange("(kc p) o -> kc p o", p=P)
    c.wT = [const.tile([P, 3 * C], BF16, tag=f"wT{kc}", name=f"wT{kc}")
            for kc in range(CK)]
    for kc in range(CK):   # whole tile: contiguous 3KB rows DMA efficiently
        nc.sync.dma_start(c.wT[kc], wT_r[kc])
    c.pT_d = pT_d
    return c


def _consts_late(nc, const, c):
    pT_r = c.pT_d.ap().rearrange("(kc p) o -> kc p o", p=P)
    c.pT = []
    for kc in range(CK):
        t = const.tile([P, C], BF16, tag=f"pT{kc}", name=f"pT{kc}")
        nc.sync.dma_start(t, pT_r[kc])
        c.pT.append(t)


def _emit(nc, tc, pools, c_box, const, x_d, out_d, wT_d, pT_d, masks_d,
          bcols_d, bmask_d, sel2_d, prow_d):
    xp, hp_, qkp, vp, ep, attp, op_, sm, csp, ps, ps2 = pools

    x_r = x_d.ap().rearrange("b (kc p) h w -> b kc p (h w)", p=P)
    o_r = out_d.ap().rearrange("b (kc p) h w -> b kc p (h w)", p=P)

    S = [Ctx() for _ in range(BPC)]

    def emit_x_dma(s):
        st_ = S[s]
        st_.x = []
        for kc in range(CK):
            xt = xp.tile([P, L], F32, tag=f"x{kc}", name=f"x{kc}_{s}")
            nc.sync.dma_start(xt[:, 0:512], x_r[s, kc][:, 0:512])
            nc.sync.dma_start(xt[:, 512:1024], x_r[s, kc][:, 512:1024])
            st_.x.append(xt)
        st_.stat2 = [None] * CK

    def emit_gn_stats_kc(s, kc):
        st_ = S[s]
        xt = st_.x[kc]
        bst = sm.tile([P, 2, 6], F32, tag="bst", name="bst")
        nc.vector.bn_stats(out=bst[:, 0, :], in_=xt[:, 0:512])
        nc.vector.bn_stats(out=bst[:, 1, :], in_=xt[:, 512:1024])
        mv = sm.tile([P, 2], F32, tag="mv", name="mv")
        nc.vector.bn_aggr(out=mv, in_=bst)
        st2 = sm.tile([P, 2], F32, tag="st2", name="st2")
        nc.vector.tensor_copy(out=st2[:, 0:1], in_=mv[:, 0:1])
        nc.vector.tensor_tensor(st2[:, 1:2], mv[:, 0:1], mv[:, 0:1], OP.mult)
        nc.vector.tensor_tensor(st2[:, 1:2], st2[:, 1:2], mv[:, 1:2], OP.add)
        st_.stat2[kc] = st2

    c = c_box

    def emit_gn_head(s):
        st_ = S[s]
        gps = ps2.tile([P, 512], F32, tag="p2", name="gn_ps")
        for kc in range(CK):
            nc.tensor.matmul(gps[0:NG, 0:2], c.gmask[kc], st_.stat2[kc],
                             start=(kc == 0), stop=(kc == CK - 1))
        gst = sm.tile([NG, 2], F32, tag="gst", name=f"gst_{s}")
        gsb = sm.tile([NG, 2], F32, tag="gsb", name="gsb")
        gtmp = sm.tile([NG, 1], F32, tag="gtmp", name="gtmp")
        nc.vector.tensor_copy(out=gsb, in_=gps[0:NG, 0:2])
        nc.vector.tensor_tensor(gtmp, gsb[:, 0:1], gsb[:, 0:1], OP.mult)
        nc.vector.tensor_tensor(gtmp, gsb[:, 1:2], gtmp, OP.subtract)  # var
        nc.scalar.activation(gtmp, gtmp, AF.Ln, bias=c.eps_t)
        nc.scalar.activation(gst[:, 1:2], gtmp, AF.Exp, scale=-0.5)    # rstd
        nc.vector.tensor_copy(out=gst[:, 0:1], in_=gsb[:, 0:1])        # gmean
        chps = ps2.tile([P, 512], F32, tag="p2", name="gn_ps2")
        for kc in range(CK):
            nc.tensor.matmul(chps[:, kc * 2: kc * 2 + 2],
                             c.bmask[:, kc * P:(kc + 1) * P], gst,
                             start=True, stop=True)
        st_.chps = chps
        # batched scale/shift columns for all 4 channel chunks: one strided
        # TT triple instead of 12 tiny DVE ops
        ch2 = chps[:, 0:2 * CK].rearrange("p (kc two) -> p two kc", two=2)
        Acols = sm.tile([P, CK], F32, tag="Acols", name=f"Acols_{s}")
        Bcols = sm.tile([P, CK], F32, tag="Bcols", name=f"Bcols_{s}")
        nc.vector.tensor_tensor(Acols, ch2[:, 1, :], c.nw_all, OP.mult)
        nc.vector.tensor_tensor(Bcols, ch2[:, 0, :], Acols, OP.mult)
        nc.vector.tensor_tensor(Bcols, c.nb_all, Bcols, OP.subtract)
        st_.Acols, st_.Bcols = Acols, Bcols
        st_.qkT = [None] * 8
        st_.v = [None] * LK
        st_.att = [None] * CK
        st_.csum = [None] * CK

    def emit_gn_h_kc(s, kc):
        st_ = S[s]
        ht = hp_.tile([P, L], BF16, tag=f"h{kc}", name=f"h{kc}_{s}")
        # pure-SBUF op; split across DVE and GpSimd so the 4 chunks finish
        # in ~2 engine-slots instead of 4
        eng = nc.vector if kc < 2 else nc.gpsimd
        eng.tensor_scalar(ht, st_.x[kc], st_.Acols[:, kc:kc + 1],
                          st_.Bcols[:, kc:kc + 1], op0=OP.mult, op1=OP.add)
        st_.h[kc] = ht

    def emit_gn_apply(s):
        S[s].h = [None] * CK
        emit_gn_head(s)
        for kc in range(CK):
            emit_gn_h_kc(s, kc)

    def emit_qkv_unit(s, oc, li):
        st_ = S[s]
        if st_.qkT[oc] is None:
            st_.qkT[oc] = qkp.tile([P, L], BF16, tag=f"qk{oc}", name=f"qk{oc}_{s}")
        dst = st_.qkT[oc]
        pt = ps2.tile([P, 512], F32, tag="p2", name="qkv_ps")
        for kc in range(CK):
            nc.tensor.matmul(pt,
                             c.wT[kc][:, oc * P:(oc + 1) * P],
                             st_.h[kc][:, li * 512:(li + 1) * 512],
                             start=(kc == 0), stop=(kc == CK - 1))
        nc.vector.tensor_scalar(dst[:, li * 512:(li + 1) * 512],
                                pt, c.qb[oc], None, op0=OP.add)

    def emit_qkv_qk(s, hp):
        for li in range(2):
            for oc in (hp, 4 + hp):
                emit_qkv_unit(s, oc, li)

    def emit_v(s, lcs):
        st_ = S[s]
        for lc in lcs:
            pt = ps2.tile([P, 512], F32, tag="p2", name="v_ps")
            for kc in range(CK):
                nc.tensor.matmul(pt,
                                 st_.h[kc][:, lc * P:(lc + 1) * P],
                                 c.wT[kc][:, 1024:1536],
                                 start=(kc == 0), stop=(kc == CK - 1))
            vt = vp.tile([P, NH, HD + 1], BF16, tag=f"v{lc}", name=f"v{lc}_{s}")
            nc.vector.tensor_copy(out=vt[:, :, HD:HD + 1],
                                  in_=c.ones_col[:, :, None])
            nc.vector.tensor_tensor(
                vt[:, :, 0:HD],
                pt.rearrange("p (h d) -> p h d", d=HD),
                c.vb.rearrange("p (h d) -> p h d", d=HD),
                OP.add)
            st_.v[lc] = vt

    fill_q = []

    def pop_fill():
        if fill_q:
            fill_q.pop(0)()

    def at_ms(ms, f):
        def g():
            with tc.tile_wait_until(ms):
                f()
        return g

    def make_norm2(s, hp, rsum):
        st_ = S[s]

        def norm2():
            for li in range(2):
                rb2 = ps2.tile([P, 512], F32, tag="p2", name="rb2_ps")
                nc.tensor.matmul(rb2, c.sel2, rsum[:, li * 512:(li + 1) * 512],
                                 start=True, stop=True)
                nc.vector.tensor_tensor(
                    st_.att[hp][:, li * 512:(li + 1) * 512],
                    st_.att[hp][:, li * 512:(li + 1) * 512], rb2, OP.mult)
        return norm2

    def emit_head(s, h):
        st_ = S[s]
        hp, h2 = h // 2, h % 2
        kT, qT = st_.qkT[4 + hp], st_.qkT[hp]
        if st_.att[hp] is None:
            st_.att[hp] = attp.tile([P, L], BF16, tag=f"att{hp}", name=f"att{hp}_{s}")
            st_.csum[hp] = csp.tile([2, L], F32, tag=f"csum{hp}",
                                    name=f"csum{hp}_{s}", bufs=1)
        av = ps.tile([P, L], F32, tag="s", name=f"av_{s}_{h}")

        def s_mm(jc):
            # matmul outputs may not cross a PSUM bank (512 f32), so the
            # full-L S^T chunk is two N=512 matmuls into one 2-bank tile
            stile = ps.tile([P, L], F32, tag="s", name=f"s_{s}_{h}_{jc}")
            for ih in range(2):
                nc.tensor.matmul(stile[:, ih * 512:(ih + 1) * 512],
                                 kT[h2 * HD:(h2 + 1) * HD, jc * P:(jc + 1) * P],
                                 qT[h2 * HD:(h2 + 1) * HD, ih * 512:(ih + 1) * 512],
                                 start=True, stop=True)
            return stile

        stile = s_mm(0)
        for jc in range(LK):
            e_t = ep.tile([P, L], BF16, tag="e", name="e_t")
            nc.scalar.activation(e_t, stile, AF.Exp, scale=SCALE)
            # emit next S ahead of this AV so the PE stream runs one step
            # ahead of ScalarE; then soak the PE with one filler unit
            if jc + 1 < LK:
                stile = s_mm(jc + 1)
            pop_fill()
            for ih in range(2):
                nc.tensor.matmul(av[0:HD + 1, ih * 512:(ih + 1) * 512],
                                 st_.v[jc][:, h, :],
                                 e_t[:, ih * 512:(ih + 1) * 512],
                                 start=(jc == 0), stop=(jc == LK - 1))
        cstage = csp.tile([1, L], F32, tag="cstage", name="cstage", bufs=2)
        nc.vector.tensor_copy(out=cstage, in_=av[HD:HD + 1, :])
        nc.sync.dma_start(st_.csum[hp][h2:h2 + 1, :], cstage)
        nc.vector.tensor_copy(out=st_.att[hp][h2 * HD:(h2 + 1) * HD, :],
                              in_=av[0:HD, :])
        if h2 == 1:
            # both heads of pair hp drained: reciprocal + bf16 cast
            rtmp = csp.tile([2, L], F32, tag="rtmp", name=f"rtmp_{s}_{hp}")
            rsum = csp.tile([2, L], BF16, tag=f"rsum{hp}",
                            name=f"rsum{hp}_{s}", bufs=1)
            nc.vector.reciprocal_approx_fast(out=rtmp, in_=st_.csum[hp])
            with nc.allow_low_precision(reason="bf16 rounding"):
                nc.vector.tensor_copy(out=rsum, in_=rtmp)
            fill_q.insert(min(len(fill_q), 4), make_norm2(s, hp, rsum))

    def emit_proj_li(s, oc, li):
        st_ = S[s]
        sl = slice(li * 512, (li + 1) * 512)
        pt = ps2.tile([P, 512], F32, tag="p2", name="proj_ps")
        nc.tensor.matmul(pt, c.pbrow[0:1, oc * P:(oc + 1) * P],
                         c.onesrow[0:1, 0:512], start=True, stop=False)
        for kc in range(CK):
            nc.tensor.matmul(pt,
                             c.pT[kc][:, oc * P:(oc + 1) * P],
                             st_.att[kc][:, sl],
                             start=False, stop=(kc == CK - 1))
        ot = op_.tile([P, 512], F32, tag="otl", name="otl")
        nc.vector.tensor_tensor(ot, pt, st_.x[oc][:, sl], OP.add)
        nc.sync.dma_start(o_r[s, oc][:, sl], ot)

    def emit_proj_oc(s, oc, split_dma=False):
        st_ = S[s]
        ot = op_.tile([P, L], F32, tag="ot", name="ot")
        for li in range(2):
            sl = slice(li * 512, (li + 1) * 512)
            pt = ps2.tile([P, 512], F32, tag="p2", name="proj_ps")
            # bias folded into the accumulation: pb_row (x) ones_row
            nc.tensor.matmul(pt, c.pbrow[0:1, oc * P:(oc + 1) * P],
                             c.onesrow[0:1, 0:512], start=True, stop=False)
            for kc in range(CK):
                nc.tensor.matmul(pt,
                                 c.pT[kc][:, oc * P:(oc + 1) * P],
                                 st_.att[kc][:, sl],
                                 start=False, stop=(kc == CK - 1))
            nc.vector.tensor_tensor(ot[:, sl], pt, st_.x[oc][:, sl], OP.add)
            if split_dma:
                nc.sync.dma_start(o_r[s, oc][:, sl], ot[:, sl])
        if not split_dma:
            nc.sync.dma_start(o_r[s, oc], ot)

    # ---------------- schedule ----------------
    emit_x_dma(0)             # x(s0) DMAs lead the queue
    cc = _consts(nc, const, wT_d, pT_d, masks_d, bcols_d, bmask_d, sel2_d,
                 prow_d)
    c.__dict__.update(cc.__dict__)
    for kc in range(CK):
        emit_gn_stats_kc(0, kc)
    emit_gn_apply(0)
    emit_qkv_qk(0, 0)         # head 0/1 q,k
    emit_x_dma(1)             # x(s1) DMAs queue behind the weight loads
    _consts_late(nc, const, c)

    # everything else becomes filler units popped one per attention step; the
    # queue order encodes just-in-time deadlines (v2(0,jcp) pops before the
    # AV pair that consumes it; qkv(0,hp) before head 2hp's first S). Sample-1
    # groupnorm runs as fillers so its DVE chain stays off the prologue.
    for lc in range(LK):
        fill_q.append(lambda lc=lc: emit_v(0, [lc]))
    for oc in (1, 5):
        for li in range(2):
            fill_q.append(lambda oc=oc, li=li: emit_qkv_unit(0, oc, li))
    for oc in (2, 6, 3, 7):
        for li in range(2):
            fill_q.append(lambda oc=oc, li=li: emit_qkv_unit(0, oc, li))
    for kc in range(CK):
        fill_q.append(lambda kc=kc: emit_gn_stats_kc(1, kc))
    fill_q.append(lambda: emit_gn_head(1))
    for kc in range(CK):
        fill_q.append(lambda kc=kc: emit_gn_h_kc(1, kc))
    for oc in (0, 4):
        for li in range(2):
            fill_q.append(lambda oc=oc, li=li: emit_qkv_unit(1, oc, li))
    for lc in range(LK):
        fill_q.append(lambda lc=lc: emit_v(1, [lc]))
    for oc in (1, 5):
        for li in range(2):
            fill_q.append(lambda oc=oc, li=li: emit_qkv_unit(1, oc, li))
    fill_q.extend([lambda: None] * 10)
    for oc in (2, 6):
        for li in range(2):
            fill_q.append(lambda oc=oc, li=li: emit_qkv_unit(1, oc, li))
    fill_q.extend([lambda: None] * 14)
    for oc in (3, 7):
        for li in range(2):
            fill_q.append(lambda oc=oc, li=li: emit_qkv_unit(1, oc, li))

    # interleave the two samples' head loops: sample-1 S/AV becomes the PE
    # filler for sample-0's exp-paced steps and vice versa, keeping the PE
    # continuously warm (HAM re-throttles on idle gaps). All h(s0) readers
    # (v(0,*), qkv(0,*)) pop before gn(1)'s h writes - else PE<->DVE cycle.
    S[1].h = [None] * CK
    seq = [(0, 0), (0, 1), (0, 2), (0, 3), (0, 4), (1, 0), (0, 5), (1, 1),
           (0, 6), (1, 2), (0, 7), (1, 3), (1, 4), (1, 5), (1, 6), (1, 7)]
    for s, h in seq:
        emit_head(s, h)
        if (s, h) == (0, 7):
            fill_q.extend([lambda: None] * 12)
            for oc in range(CK):
                for li in range(2):
                    fill_q.append(lambda oc=oc, li=li: emit_proj_li(0, oc, li))
                    fill_q.append(lambda: None)
    while fill_q:
        pop_fill()
    for oc in range(CK):
        for li in range(2):
            emit_proj_li(1, oc, li)


def _build():
    if "nc" in _NC_CACHE:
        return _NC_CACHE["nc"]
    nc = bacc.Bacc("TRN2", target_bir_lowering=False, debug=False)
    x_d = nc.dram_tensor("x", (BPC, C, H, W), F32, kind="ExternalInput")
    wT_d = nc.dram_tensor("wT", (C, 3 * C), BF16, kind="ExternalInput")
    pT_d = nc.dram_tensor("pT", (C, C), BF16, kind="ExternalInput")
    masks_d = nc.dram_tensor("masks", (P, 512 + CK * NG + NH), F32,
                             kind="ExternalInput")
    bcols_d = nc.dram_tensor("bcols", (P, 20), F32, kind="ExternalInput")
    bmask_d = nc.dram_tensor("bmask", (NG, C), F32, kind="ExternalInput")
    sel2_d = nc.dram_tensor("sel2", (2, P), BF16, kind="ExternalInput")
    prow_d = nc.dram_tensor("prow", (2, C), BF16, kind="ExternalInput")
    out_d = nc.dram_tensor("out", (BPC, C, H, W), F32, kind="ExternalOutput")
    with tile.TileContext(nc) as tc:
        with (
            tc.tile_pool(name="const", bufs=1) as const,
            tc.tile_pool(name="xp", bufs=2) as xp,
            tc.tile_pool(name="hp", bufs=1) as hp_,
            tc.tile_pool(name="qkp", bufs=2) as qkp,
            tc.tile_pool(name="vp", bufs=2) as vp,
            tc.tile_pool(name="ep", bufs=3) as ep,
            tc.tile_pool(name="attp", bufs=2) as attp,
            tc.tile_pool(name="op", bufs=2) as op_,
            tc.tile_pool(name="sm", bufs=1) as sm,
            tc.tile_pool(name="csp", bufs=2) as csp,
            tc.tile_pool(name="ps", bufs=3, space="PSUM") as ps,
            tc.tile_pool(name="ps2", bufs=2, space="PSUM") as ps2,
        ):
            pools = (xp, hp_, qkp, vp, ep, attp, op_, sm, csp, ps, ps2)
            _emit(nc, tc, pools, Ctx(), const, x_d, out_d, wT_d, pT_d,
                  masks_d, bcols_d, bmask_d, sel2_d, prow_d)
    nc.compile()
    _NC_CACHE["nc"] = nc
    return nc


def _host_consts(norm_w, norm_b, qkv_w, qkv_b, proj_w, proj_b):
    bf16 = ml_dtypes.bfloat16
    wT = np.ascontiguousarray(qkv_w.T).astype(bf16)
    pT = np.ascontiguousarray(proj_w.T).astype(bf16)

    masks = np.zeros((P, 512 + CK * NG + NH), np.float32)
    masks[:, 0:512] = qkv_b[1024:1536][None, :]                      # vb
    for kc in range(CK):
        for p in range(P):
            g = (kc * P + p) // GS
            masks[p, 512 + kc * NG + g] = 1.0 / GS                   # gmask
    masks[:, 512 + CK * NG:] = 1.0                                   # ones

    bcols = np.zeros((P, 20), np.float32)
    for kc in range(CK):
        bcols[:, kc] = norm_w[kc * P:(kc + 1) * P]
        bcols[:, 4 + kc] = norm_b[kc * P:(kc + 1) * P]
        bcols[:, 8 + kc] = proj_b[kc * P:(kc + 1) * P]
    for oc in range(8):
        bcols[:, 12 + oc] = qkv_b[oc * P:(oc + 1) * P]

    bmask = np.zeros((NG, C), np.float32)
    for g in range(NG):
        bmask[g, g * GS:(g + 1) * GS] = 1.0

    sel2 = np.zeros((2, P), np.float32)
    sel2[0, 0:HD] = 1.0
    sel2[1, HD:P] = 1.0

    prow = np.ones((2, C), np.float32)
    prow[0] = proj_b

    return {"wT": wT, "pT": pT, "masks": masks, "bcols": bcols,
            "bmask": bmask, "sel2": sel2.astype(bf16),
            "prow": prow.astype(bf16)}


def make_in_maps(x, norm_w, norm_b, qkv_w, qkv_b, proj_w, proj_b):
    x = np.ascontiguousarray(x, dtype=np.float32)
    args = _host_consts(
        np.asarray(norm_w, np.float32), np.asarray(norm_b, np.float32),
        np.ascontiguousarray(qkv_w, np.float32), np.asarray(qkv_b, np.float32),
        np.ascontiguousarray(proj_w, np.float32), np.asarray(proj_b, np.float32))
    return [dict(args, x=x[i * BPC:(i + 1) * BPC]) for i in range(N_CORES)]


def kernel(x, norm_w, norm_b, qkv_w, qkv_b, proj_w, proj_b):
    nc = _build()
    in_maps = make_in_maps(x, norm_w, norm_b, qkv_w, qkv_b, proj_w, proj_b)
    res = run_bass_kernel_spmd(nc, in_maps, core_ids=list(range(N_CORES)))
    return np.concatenate([r["out"] for r in res.results], axis=0)


# revision 58
# speedup vs baseline: 1.1491x; 1.0127x over previous
"""AttentionBlock (GroupNorm + 8-head self-attention + proj + residual) on 8 trn2 cores.

Sharding: data-parallel over batch B=16 -> 2 samples per core. No collectives.

v2 (per-head bf16 pipeline, cross-sample interleaved):
  - All PE matmul operands are bf16 (hardware runs fp32r as two half-rate
    LOW_HIGH passes; bf16 is single-pass and halves LDWEIGHTS). PSUM fp32.
  - Weights arrive HOST-pretransposed and pre-cast: wT = qkv_w.T (bf16),
    pT = proj_w.T (bf16), plus precomputed groupnorm/selector masks and
    bias columns - no on-device transposes or gpsimd mask building.
  - GroupNorm per sample: bn_stats over L, 16-channel group aggregation +
    broadcast-back via tiny mask matmuls on the PE (fp32, negligible);
    normalized h written by DVE/GpSimd tensor_scalar (bf16).
  - QKV: q^T,k^T as (channels, L) bf16 tiles; v in (L, channels) orientation
    with a ones column appended (M=65) so the softmax denominator rides in
    PSUM row 64 of the AV matmul.
  - Attention PER HEAD: per j-chunk, S^T = kT.T @ qT as two N=512 matmuls
    into one 2-bank PSUM tile (matmul out must not cross a PSUM bank), ONE
    [128,1024] exp on ScalarE (scale 1/8 fused) -> bf16 e tile, AV
    accumulates v'[jc].T @ e into a (65, 1024) PSUM tile.
  - Per pair: denominators to a (2, L) tile (DVE copy + DMA), then
    reciprocal_approx_fast + bf16 cast; a K=2 selector matmul broadcasts
    them to channel rows for one normalization multiply.
  - proj with bias folded into the accumulation as a rank-1 matmul,
    residual from SBUF-resident x, write out.
  - The two samples' head loops INTERLEAVE (s0 h0-4, then alternating),
    so each sample's S/AV fills the other's exp-paced stalls: the PE stays
    continuously busy and HAM keeps the clock up. Remaining work (qkv/v of
    sample 1, proj of sample 0, sample-1 groupnorm) pops one unit per
    attention step from a deadline-ordered filler queue with spacers to
    reach the otherwise-starved late steps.
"""

import numpy as np
import ml_dtypes

import concourse.bass as bass
import concourse.mybir as mybir
import concourse.tile as tile
from concourse import bacc
from concourse.bass_utils import run_bass_kernel_spmd

F32 = mybir.dt.float32
BF16 = mybir.dt.bfloat16
FP8 = mybir.dt.float8e4
DR = mybir.MatmulPerfMode.DoubleRow
EXP_BIAS = -2.0  # exp(s-2): keeps e in fp8e4m3 range; cancels in the softmax ratio
AF = mybir.ActivationFunctionType
OP = mybir.AluOpType

B, C, H, W = 16, 512, 32, 32
L = H * W
NH, HD = 8, 64
NG, GS = 32, 16
EPS = 1e-5
N_CORES = 8
BPC = B // N_CORES  # samples per core
P = 128
CK = C // P   # 4 channel chunks
LK = L // P   # 8 pixel chunks
SCALE = HD ** -0.5

_NC_CACHE = {}


class Ctx:
    pass


def _consts(nc, const, wT_d, pT_d, masks_d, bcols_d, bmask_d, sel2_d, prow_d):
    c_prow_ap = prow_d.ap()
    """Emit const DMAs in deadline order: small gn masks first, then the
    weight columns the prologue touches (q0/k0 chunks, v), then the rest.
    pT is deferred to _consts_late (first needed mid-attention)."""
    c = Ctx()

    # masks layout: [vb (512) | gmask (4*32) | ones (8)]
    masks = const.tile([P, 512 + CK * NG + NH], F32, tag="masks", name="masks")
    nc.sync.dma_start(masks, masks_d.ap())
    c.vb = masks[:, 0:512]
    c.gmask = [masks[:, 512 + kc * NG: 512 + (kc + 1) * NG] for kc in range(CK)]
    c.ones_col = masks[:, 512 + CK * NG: 512 + CK * NG + NH]

    # bcols layout: [nw (4) | nb (4) | pb (4) | qb (8)]
    bcols = const.tile([P, 20], F32, tag="bcols", name="bcols")
    nc.sync.dma_start(bcols, bcols_d.ap())
    c.nw = [bcols[:, kc: kc + 1] for kc in range(CK)]
    c.nb = [bcols[:, 4 + kc: 5 + kc] for kc in range(CK)]
    c.nw_all = bcols[:, 0:CK]
    c.nb_all = bcols[:, 4:4 + CK]
    c.pb = [bcols[:, 8 + kc: 9 + kc] for kc in range(CK)]
    c.qb = [bcols[:, 12 + oc: 13 + oc] for oc in range(8)]

    c.bmask = const.tile([NG, C], F32, tag="bmask", name="bmask")
    nc.sync.dma_start(c.bmask, bmask_d.ap())
    c.sel2 = const.tile([2, P], BF16, tag="sel2", name="sel2")
    nc.sync.dma_start(c.sel2, sel2_d.ap())
    c.pbrow = const.tile([1, C], BF16, tag="pbrow", name="pbrow")
    nc.sync.dma_start(c.pbrow, c_prow_ap[0:1])
    c.onesrow = const.tile([1, C], BF16, tag="onesrow", name="onesrow")
    nc.sync.dma_start(c.onesrow, c_prow_ap[1:2])
    c.eps_t = const.tile([NG, 1], F32, tag="eps_t")
    nc.vector.memset(c.eps_t, EPS)
    c.ebias = const.tile([P, 1], F32, tag="ebias")
    nc.vector.memset(c.ebias, EXP_BIAS)

    wT_r = wT_d.ap().rearrange("(kc p) o -> kc p o", p=P)
    c.wT = [const.tile([P, 3 * C], BF16, tag=f"wT{kc}", name=f"wT{kc}")
            for kc in range(CK)]
    for kc in range(CK):   # whole tile: contiguous 3KB rows DMA efficiently
        nc.sync.dma_start(c.wT[kc], wT_r[kc])
    c.pT_d = pT_d
    return c


def _consts_late(nc, const, c):
    pT_r = c.pT_d.ap().rearrange("(kc p) o -> kc p o", p=P)
    c.pT = []
    for kc in range(CK):
        t = const.tile([P, C], BF16, tag=f"pT{kc}", name=f"pT{kc}")
        nc.sync.dma_start(t, pT_r[kc])
        c.pT.append(t)


def _emit(nc, tc, pools, c_box, const, x_d, out_d, wT_d, pT_d, masks_d,
          bcols_d, bmask_d, sel2_d, prow_d):
    xp, hp_, qkp, vp, ep, attp, op_, sm, csp, ps, ps2 = pools

    x_r = x_d.ap().rearrange("b (kc p) h w -> b kc p (h w)", p=P)
    o_r = out_d.ap().rearrange("b (kc p) h w -> b kc p (h w)", p=P)

    S = [Ctx() for _ in range(BPC)]

    def emit_x_dma(s):
        st_ = S[s]
        st_.x = []
        for kc in range(CK):
            xt = xp.tile([P, L], F32, tag=f"x{kc}", name=f"x{kc}_{s}")
            nc.sync.dma_start(xt[:, 0:512], x_r[s, kc][:, 0:512])
            nc.sync.dma_start(xt[:, 512:1024], x_r[s, kc][:, 512:1024])
            st_.x.append(xt)
        st_.stat2 = [None] * CK

    def emit_gn_stats_kc(s, kc):
        st_ = S[s]
        xt = st_.x[kc]
        bst = sm.tile([P, 2, 6], F32, tag="bst", name="bst")
        nc.vector.bn_stats(out=bst[:, 0, :], in_=xt[:, 0:512])
        nc.vector.bn_stats(out=bst[:, 1, :], in_=xt[:, 512:1024])
        mv = sm.tile([P, 2], F32, tag="mv", name="mv")
        nc.vector.bn_aggr(out=mv, in_=bst)
        st2 = sm.tile([P, 2], F32, tag="st2", name="st2")
        nc.vector.tensor_copy(out=st2[:, 0:1], in_=mv[:, 0:1])
        nc.vector.tensor_tensor(st2[:, 1:2], mv[:, 0:1], mv[:, 0:1], OP.mult)
        nc.vector.tensor_tensor(st2[:, 1:2], st2[:, 1:2], mv[:, 1:2], OP.add)
        st_.stat2[kc] = st2

    c = c_box

    def emit_gn_head(s):
        st_ = S[s]
        gps = ps2.tile([P, 512], F32, tag="p2", name="gn_ps")
        for kc in range(CK):
            nc.tensor.matmul(gps[0:NG, 0:2], c.gmask[kc], st_.stat2[kc],
                             start=(kc == 0), stop=(kc == CK - 1))
        gst = sm.tile([NG, 2], F32, tag="gst", name=f"gst_{s}")
        gsb = sm.tile([NG, 2], F32, tag="gsb", name="gsb")
        gtmp = sm.tile([NG, 1], F32, tag="gtmp", name="gtmp")
        nc.vector.tensor_copy(out=gsb, in_=gps[0:NG, 0:2])
        nc.vector.tensor_tensor(gtmp, gsb[:, 0:1], gsb[:, 0:1], OP.mult)
        nc.vector.tensor_tensor(gtmp, gsb[:, 1:2], gtmp, OP.subtract)  # var
        nc.scalar.activation(gtmp, gtmp, AF.Ln, bias=c.eps_t)
        nc.scalar.activation(gst[:, 1:2], gtmp, AF.Exp, scale=-0.5)    # rstd
        nc.vector.tensor_copy(out=gst[:, 0:1], in_=gsb[:, 0:1])        # gmean
        chps = ps2.tile([P, 512], F32, tag="p2", name="gn_ps2")
        for kc in range(CK):
            nc.tensor.matmul(chps[:, kc * 2: kc * 2 + 2],
                             c.bmask[:, kc * P:(kc + 1) * P], gst,
                             start=True, stop=True)
        st_.chps = chps
        # batched scale/shift columns for all 4 channel chunks: one strided
        # TT triple instead of 12 tiny DVE ops
        ch2 = chps[:, 0:2 * CK].rearrange("p (kc two) -> p two kc", two=2)
        Acols = sm.tile([P, CK], F32, tag="Acols", name=f"Acols_{s}")
        Bcols = sm.tile([P, CK], F32, tag="Bcols", name=f"Bcols_{s}")
        nc.vector.tensor_tensor(Acols, ch2[:, 1, :], c.nw_all, OP.mult)
        nc.vector.tensor_tensor(Bcols, ch2[:, 0, :], Acols, OP.mult)
        nc.vector.tensor_tensor(Bcols, c.nb_all, Bcols, OP.subtract)
        st_.Acols, st_.Bcols = Acols, Bcols
        st_.qkT = [None] * 8
        st_.v = [None] * LK
        st_.att = [None] * CK
        st_.csum = [None] * CK

    def emit_gn_h_kc(s, kc):
        st_ = S[s]
        ht = hp_.tile([P, L], BF16, tag=f"h{kc}", name=f"h{kc}_{s}")
        # pure-SBUF op; split across DVE and GpSimd so the 4 chunks finish
        # in ~2 engine-slots instead of 4
        eng = nc.vector if kc < 2 else nc.gpsimd
        eng.tensor_scalar(ht, st_.x[kc], st_.Acols[:, kc:kc + 1],
                          st_.Bcols[:, kc:kc + 1], op0=OP.mult, op1=OP.add)
        st_.h[kc] = ht

    def emit_gn_apply(s):
        S[s].h = [None] * CK
        emit_gn_head(s)
        for kc in range(CK):
            emit_gn_h_kc(s, kc)

    def emit_qkv_unit(s, oc, li):
        st_ = S[s]
        if st_.qkT[oc] is None:
            st_.qkT[oc] = qkp.tile([P, L], BF16, tag=f"qk{oc}", name=f"qk{oc}_{s}")
        dst = st_.qkT[oc]
        pt = ps2.tile([P, 512], F32, tag="p2", name="qkv_ps")
        for kc in range(CK):
            nc.tensor.matmul(pt,
                             c.wT[kc][:, oc * P:(oc + 1) * P],
                             st_.h[kc][:, li * 512:(li + 1) * 512],
                             start=(kc == 0), stop=(kc == CK - 1))
        nc.vector.tensor_scalar(dst[:, li * 512:(li + 1) * 512],
                                pt, c.qb[oc], None, op0=OP.add)

    def emit_qkv_qk(s, hp):
        for li in range(2):
            for oc in (hp, 4 + hp):
                emit_qkv_unit(s, oc, li)

    def emit_v(s, lcs):
        st_ = S[s]
        for lc in lcs:
            pt = ps2.tile([P, 512], F32, tag="p2", name="v_ps")
            for kc in range(CK):
                nc.tensor.matmul(pt,
                                 st_.h[kc][:, lc * P:(lc + 1) * P],
                                 c.wT[kc][:, 1024:1536],
                                 start=(kc == 0), stop=(kc == CK - 1))
            vt = vp.tile([P, NH, HD + 1], BF16, tag=f"v{lc}", name=f"v{lc}_{s}")
            nc.vector.tensor_copy(out=vt[:, :, HD:HD + 1],
                                  in_=c.ones_col[:, :, None])
            nc.vector.tensor_tensor(
                vt[:, :, 0:HD],
                pt.rearrange("p (h d) -> p h d", d=HD),
                c.vb.rearrange("p (h d) -> p h d", d=HD),
                OP.add)
            st_.v[lc] = vt

    fill_q = []

    def pop_fill():
        if fill_q:
            fill_q.pop(0)()

    def at_ms(ms, f):
        def g():
            with tc.tile_wait_until(ms):
                f()
        return g

    def make_norm2(s, hp, rsum):
        st_ = S[s]

        def norm2():
            for li in range(2):
                rb2 = ps2.tile([P, 512], F32, tag="p2", name="rb2_ps")
                nc.tensor.matmul(rb2, c.sel2, rsum[:, li * 512:(li + 1) * 512],
                                 start=True, stop=True)
                nc.vector.tensor_tensor(
                    st_.att[hp][:, li * 512:(li + 1) * 512],
                    st_.att[hp][:, li * 512:(li + 1) * 512], rb2, OP.mult)
        return norm2

    def emit_head(s, h):
        st_ = S[s]
        hp, h2 = h // 2, h % 2
        kT, qT = st_.qkT[4 + hp], st_.qkT[hp]
        if st_.att[hp] is None:
            st_.att[hp] = attp.tile([P, L], BF16, tag=f"att{hp}", name=f"att{hp}_{s}")
            st_.csum[hp] = csp.tile([2, L], F32, tag=f"csum{hp}",
                                    name=f"csum{hp}_{s}", bufs=1)
        av = ps.tile([P, L], F32, tag="s", name=f"av_{s}_{h}")

        def s_mm(jc):
            # matmul outputs may not cross a PSUM bank (512 f32), so the
            # full-L S^T chunk is two N=512 matmuls into one 2-bank tile
            stile = ps.tile([P, L], F32, tag="s", name=f"s_{s}_{h}_{jc}")
            for ih in range(2):
                nc.tensor.matmul(stile[:, ih * 512:(ih + 1) * 512],
                                 kT[h2 * HD:(h2 + 1) * HD, jc * P:(jc + 1) * P],
                                 qT[h2 * HD:(h2 + 1) * HD, ih * 512:(ih + 1) * 512],
                                 start=True, stop=True)
            return stile

        stile = s_mm(0)
        for jc in range(LK):
            e_t = ep.tile([P, L], BF16, tag="e", name="e_t")
            nc.scalar.activation(e_t, stile, AF.Exp, scale=SCALE)
            # emit next S ahead of this AV so the PE stream runs one step
            # ahead of ScalarE; then soak the PE with one filler unit
            if jc + 1 < LK:
                stile = s_mm(jc + 1)
            pop_fill()
            for ih in range(2):
                nc.tensor.matmul(av[0:HD + 1, ih * 512:(ih + 1) * 512],
                                 st_.v[jc][:, h, :],
                                 e_t[:, ih * 512:(ih + 1) * 512],
                                 start=(jc == 0), stop=(jc == LK - 1))
        cstage = csp.tile([1, L], F32, tag="cstage", name="cstage", bufs=2)
        nc.vector.tensor_copy(out=cstage, in_=av[HD:HD + 1, :])
        nc.sync.dma_start(st_.csum[hp][h2:h2 + 1, :], cstage)
        nc.vector.tensor_copy(out=st_.att[hp][h2 * HD:(h2 + 1) * HD, :],
                              in_=av[0:HD, :])
        if h2 == 1:
            # both heads of pair hp drained: reciprocal + bf16 cast
            rtmp = csp.tile([2, L], F32, tag="rtmp", name=f"rtmp_{s}_{hp}")
            rsum = csp.tile([2, L], BF16, tag=f"rsum{hp}",
                            name=f"rsum{hp}_{s}", bufs=1)
            nc.vector.reciprocal_approx_fast(out=rtmp, in_=st_.csum[hp])
            with nc.allow_low_precision(reason="bf16 rounding"):
                nc.vector.tensor_copy(out=rsum, in_=rtmp)
            fill_q.insert(min(len(fill_q), 4), make_norm2(s, hp, rsum))

    def emit_proj_li(s, oc, li):
        st_ = S[s]
        sl = slice(li * 512, (li + 1) * 512)
        pt = ps2.tile([P, 512], F32, tag="p2", name="proj_ps")
        nc.tensor.matmul(pt, c.pbrow[0:1, oc * P:(oc + 1) * P],
                         c.onesrow[0:1, 0:512], start=True, stop=False)
        for kc in range(CK):
            nc.tensor.matmul(pt,
                             c.pT[kc][:, oc * P:(oc + 1) * P],
                             st_.att[kc][:, sl],
                             start=False, stop=(kc == CK - 1))
        ot = op_.tile([P, 512], F32, tag="otl", name="otl")
        nc.vector.tensor_tensor(ot, pt, st_.x[oc][:, sl], OP.add)
        nc.sync.dma_start(o_r[s, oc][:, sl], ot)

    def emit_proj_oc(s, oc, split_dma=False):
        st_ = S[s]
        ot = op_.tile([P, L], F32, tag="ot", name="ot")
        for li in range(2):
            sl = slice(li * 512, (li + 1) * 512)
            pt = ps2.tile([P, 512], F32, tag="p2", name="proj_ps")
            # bias folded into the accumulation: pb_row (x) ones_row
            nc.tensor.matmul(pt, c.pbrow[0:1, oc * P:(oc + 1) * P],
                             c.onesrow[0:1, 0:512], start=True, stop=False)
            for kc in range(CK):
                nc.tensor.matmul(pt,
                                 c.pT[kc][:, oc * P:(oc + 1) * P],
                                 st_.att[kc][:, sl],
                                 start=False, stop=(kc == CK - 1))
            nc.vector.tensor_tensor(ot[:, sl], pt, st_.x[oc][:, sl], OP.add)
            if split_dma:
                nc.sync.dma_start(o_r[s, oc][:, sl], ot[:, sl])
        if not split_dma:
            nc.sync.dma_start(o_r[s, oc], ot)

    # ---------------- schedule ----------------
    emit_x_dma(0)             # x(s0) DMAs lead the queue
    cc = _consts(nc, const, wT_d, pT_d, masks_d, bcols_d, bmask_d, sel2_d,
                 prow_d)
    c.__dict__.update(cc.__dict__)
    for kc in range(CK):
        emit_gn_stats_kc(0, kc)
    emit_gn_apply(0)
    emit_qkv_qk(0, 0)         # head 0/1 q,k
    emit_x_dma(1)             # x(s1) DMAs queue behind the weight loads
    _consts_late(nc, const, c)

    # everything else becomes filler units popped one per attention step; the
    # queue order encodes just-in-time deadlines (v2(0,jcp) pops before the
    # AV pair that consumes it; qkv(0,hp) before head 2hp's first S). Sample-1
    # groupnorm runs as fillers so its DVE chain stays off the prologue.
    for lc in range(LK):
        fill_q.append(lambda lc=lc: emit_v(0, [lc]))
    gn1 = [lambda kc=kc: emit_gn_stats_kc(1, kc) for kc in range(CK)]
    for oc in (1, 5):
        for li in range(2):
            fill_q.append(lambda oc=oc, li=li: emit_qkv_unit(0, oc, li))
            if gn1:
                fill_q.append(gn1.pop(0))
    for oc in (2, 6, 3, 7):
        for li in range(2):
            fill_q.append(lambda oc=oc, li=li: emit_qkv_unit(0, oc, li))
    fill_q.append(lambda: emit_gn_head(1))
    for kc in range(CK):
        fill_q.append(lambda kc=kc: emit_gn_h_kc(1, kc))
    for oc in (0, 4):
        for li in range(2):
            fill_q.append(lambda oc=oc, li=li: emit_qkv_unit(1, oc, li))
    for lc in range(LK):
        fill_q.append(lambda lc=lc: emit_v(1, [lc]))
    for oc in (1, 5):
        for li in range(2):
            fill_q.append(lambda oc=oc, li=li: emit_qkv_unit(1, oc, li))
    fill_q.extend([lambda: None] * 10)
    for oc in (2, 6):
        for li in range(2):
            fill_q.append(lambda oc=oc, li=li: emit_qkv_unit(1, oc, li))
    fill_q.extend([lambda: None] * 14)
    for oc in (3, 7):
        for li in range(2):
            fill_q.append(lambda oc=oc, li=li: emit_qkv_unit(1, oc, li))

    # interleave the two samples' head loops: sample-1 S/AV becomes the PE
    # filler for sample-0's exp-paced steps and vice versa, keeping the PE
    # continuously warm (HAM re-throttles on idle gaps). All h(s0) readers
    # (v(0,*), qkv(0,*)) pop before gn(1)'s h writes - else PE<->DVE cycle.
    S[1].h = [None] * CK
    seq = [(0, 0), (0, 1), (0, 2), (0, 3), (0, 4), (1, 0), (0, 5), (1, 1),
           (0, 6), (1, 2), (0, 7), (1, 3), (1, 4), (1, 5), (1, 6), (1, 7)]
    for s, h in seq:
        emit_head(s, h)
        if (s, h) == (0, 7):
            fill_q.extend([lambda: None] * 12)
            for oc in range(CK):
                for li in range(2):
                    fill_q.append(lambda oc=oc, li=li: emit_proj_li(0, oc, li))
                    fill_q.append(lambda: None)
    while fill_q:
        pop_fill()
    for oc in range(CK):
        for li in range(2):
            emit_proj_li(1, oc, li)


def _build():
    if "nc" in _NC_CACHE:
        return _NC_CACHE["nc"]
    nc = bacc.Bacc("TRN2", target_bir_lowering=False, debug=False)
    x_d = nc.dram_tensor("x", (BPC, C, H, W), F32, kind="ExternalInput")
    wT_d = nc.dram_tensor("wT", (C, 3 * C), BF16, kind="ExternalInput")
    pT_d = nc.dram_tensor("pT", (C, C), BF16, kind="ExternalInput")
    masks_d = nc.dram_tensor("masks", (P, 512 + CK * NG + NH), F32,
                             kind="ExternalInput")
    bcols_d = nc.dram_tensor("bcols", (P, 20), F32, kind="ExternalInput")
    bmask_d = nc.dram_tensor("bmask", (NG, C), F32, kind="ExternalInput")
    sel2_d = nc.dram_tensor("sel2", (2, P), BF16, kind="ExternalInput")
    prow_d = nc.dram_tensor("prow", (2, C), BF16, kind="ExternalInput")
    out_d = nc.dram_tensor("out", (BPC, C, H, W), F32, kind="ExternalOutput")
    with tile.TileContext(nc) as tc:
        with (
            tc.tile_pool(name="const", bufs=1) as const,
            tc.tile_pool(name="xp", bufs=2) as xp,
            tc.tile_pool(name="hp", bufs=1) as hp_,
            tc.tile_pool(name="qkp", bufs=2) as qkp,
            tc.tile_pool(name="vp", bufs=2) as vp,
            tc.tile_pool(name="ep", bufs=4) as ep,
            tc.tile_pool(name="attp", bufs=2) as attp,
            tc.tile_pool(name="op", bufs=2) as op_,
            tc.tile_pool(name="sm", bufs=1) as sm,
            tc.tile_pool(name="csp", bufs=2) as csp,
            tc.tile_pool(name="ps", bufs=3, space="PSUM") as ps,
            tc.tile_pool(name="ps2", bufs=2, space="PSUM") as ps2,
        ):
            pools = (xp, hp_, qkp, vp, ep, attp, op_, sm, csp, ps, ps2)
            _emit(nc, tc, pools, Ctx(), const, x_d, out_d, wT_d, pT_d,
                  masks_d, bcols_d, bmask_d, sel2_d, prow_d)
    nc.compile()
    _NC_CACHE["nc"] = nc
    return nc


def _host_consts(norm_w, norm_b, qkv_w, qkv_b, proj_w, proj_b):
    bf16 = ml_dtypes.bfloat16
    wT = np.ascontiguousarray(qkv_w.T).astype(bf16)
    pT = np.ascontiguousarray(proj_w.T).astype(bf16)

    masks = np.zeros((P, 512 + CK * NG + NH), np.float32)
    masks[:, 0:512] = qkv_b[1024:1536][None, :]                      # vb
    for kc in range(CK):
        for p in range(P):
            g = (kc * P + p) // GS
            masks[p, 512 + kc * NG + g] = 1.0 / GS                   # gmask
    masks[:, 512 + CK * NG:] = 1.0                                   # ones

    bcols = np.zeros((P, 20), np.float32)
    for kc in range(CK):
        bcols[:, kc] = norm_w[kc * P:(kc + 1) * P]
        bcols[:, 4 + kc] = norm_b[kc * P:(kc + 1) * P]
        bcols[:, 8 + kc] = proj_b[kc * P:(kc + 1) * P]
    for oc in range(8):
        bcols[:, 12 + oc] = qkv_b[oc * P:(oc + 1) * P]

    bmask = np.zeros((NG, C), np.float32)
    for g in range(NG):
        bmask[g, g * GS:(g + 1) * GS] = 1.0

    sel2 = np.zeros((2, P), np.float32)
    sel2[0, 0:HD] = 1.0
    sel2[1, HD:P] = 1.0

    prow = np.ones((2, C), np.float32)
    prow[0] = proj_b

    return {"wT": wT, "pT": pT, "masks": masks, "bcols": bcols,
            "bmask": bmask, "sel2": sel2.astype(bf16),
            "prow": prow.astype(bf16)}


def make_in_maps(x, norm_w, norm_b, qkv_w, qkv_b, proj_w, proj_b):
    x = np.ascontiguousarray(x, dtype=np.float32)
    args = _host_consts(
        np.asarray(norm_w, np.float32), np.asarray(norm_b, np.float32),
        np.ascontiguousarray(qkv_w, np.float32), np.asarray(qkv_b, np.float32),
        np.ascontiguousarray(proj_w, np.float32), np.asarray(proj_b, np.float32))
    return [dict(args, x=x[i * BPC:(i + 1) * BPC]) for i in range(N_CORES)]


def kernel(x, norm_w, norm_b, qkv_w, qkv_b, proj_w, proj_b):
    nc = _build()
    in_maps = make_in_maps(x, norm_w, norm_b, qkv_w, qkv_b, proj_w, proj_b)
    res = run_bass_kernel_spmd(nc, in_maps, core_ids=list(range(N_CORES)))
    return np.concatenate([r["out"] for r in res.results], axis=0)
